# revision 5
# baseline (speedup 1.0000x reference)
"""Causal flash attention for Trainium2, sharded 2 heads/core over 8 cores.

Math per head: out = softmax_causal(Q K^T / sqrt(D)) @ V,  Q/K/V [S=2048, D=64] fp32.

Device layout (per core, heads h0=2c, h1=2c+1):
  qT   [128, 2048]  rows 64h+d = Q[h]^T        (D on partitions, both heads stacked)
  kT   [128, 2048]  same for K
  vaug [2, 128, 1040] vaug[h, p, 65*kc+d] = V[h, 128*kc+p, d], d=64 column is ones
  outT [2, 64, 2048]  out[h]^T (normalized)

Scores are computed transposed (S^T[k, q] = K_chunk @ Q^T) so no transposes are
needed anywhere: softmax denominator comes out of the PV matmul via the ones
column of vaug (psum row 64), and the final division is done by broadcasting
1/denom across partitions with a K=1 matmul against a ones vector.
"""

import os
import sys

import numpy as np

sys.path.insert(0, "/opt/trn_rl_repo")

import concourse.bass as bass
import concourse.bacc as bacc
import concourse.mybir as mybir
import concourse.tile as tile
from concourse.bass_utils import run_bass_kernel_spmd

B, H, S, D = 1, 16, 2048, 64
N_CORES = 8
HEADS_PER_CORE = H // N_CORES  # 2
N_CHUNKS = S // 128  # 16 key chunks per head
N_SPANS = S // 512  # 4 query spans per head
F32 = mybir.dt.float32
NEG = -1.0e30

_NC = None
_LAST_RESULTS = None


def _build_bass():
    nc = bacc.Bacc("TRN2", target_bir_lowering=False)
    qT = nc.declare_dram_parameter("qT", [128, S], F32, isOutput=False)
    kT = nc.declare_dram_parameter("kT", [128, S], F32, isOutput=False)
    vaug = nc.declare_dram_parameter("vaug", [2, 128, 65 * N_CHUNKS], F32, isOutput=False)
    outT = nc.declare_dram_parameter("outT", [2, 64, S], F32, isOutput=True)

    with tile.TileContext(nc) as tc:
        with (
            tc.tile_pool(name="const", bufs=1) as const,
            tc.tile_pool(name="inbuf", bufs=1) as inbuf,
            tc.tile_pool(name="pbuf", bufs=3) as pbuf,
            tc.tile_pool(name="nbuf", bufs=3) as nbuf,
            tc.tile_pool(name="ps_s", bufs=2, space="PSUM") as ps_s,
            tc.tile_pool(name="ps_o", bufs=2, space="PSUM") as ps_o,
            tc.tile_pool(name="ps_r", bufs=2, space="PSUM") as ps_r,
        ):
            # Additive causal mask for a diagonal 128x128 block:
            # keep score where q >= k, else -1e30.
            mask_sb = const.tile([128, 128], F32)
            nc.gpsimd.memset(mask_sb, 0.0)
            nc.gpsimd.affine_select(
                out=mask_sb,
                in_=mask_sb,
                compare_op=mybir.AluOpType.is_ge,
                fill=NEG,
                base=0,
                pattern=[[1, 128]],  # +q per free element
                channel_multiplier=-1,  # -k per partition
            )
            # Ones vector on partition 64 (K=1 lhsT for the 1/denom broadcast
            # matmul; partition must match the denom row of psum_o).
            ones_sb = const.tile([65, 64], F32)
            nc.vector.memset(ones_sb, 1.0)

            # Input loads, chunked by 512 query/key columns so compute can
            # start before all DMAs land.
            qsb = []
            ksb = []
            for j in range(N_SPANS):
                qt = inbuf.tile([128, 512], F32, tag=f"q{j}")
                nc.sync.dma_start(out=qt, in_=qT[:, 512 * j : 512 * (j + 1)])
                qsb.append(qt)
                kt = inbuf.tile([128, 512], F32, tag=f"k{j}")
                nc.sync.dma_start(out=kt, in_=kT[:, 512 * j : 512 * (j + 1)])
                ksb.append(kt)
            vsb = []
            for h in range(2):
                row = []
                for j in range(N_SPANS):
                    vt = inbuf.tile([128, 4 * 65], F32, tag=f"v{h}{j}")
                    nc.sync.dma_start(
                        out=vt, in_=vaug[h, :, 260 * j : 260 * (j + 1)]
                    )
                    row.append(vt)
                vsb.append(row)

            def k_slice(h, kc):
                # kT chunk [64, 128] for head h: lhsT of the scores matmul.
                return ksb[kc // 4][64 * h : 64 * h + 64, 128 * (kc % 4) : 128 * (kc % 4) + 128]

            def q_slice(h, qs, qe):
                j = qs // 512
                base = 512 * j
                return qsb[j][64 * h : 64 * h + 64, qs - base : qe - base]

            def v_slice(h, kc):
                return vsb[h][kc // 4][:, 65 * (kc % 4) : 65 * (kc % 4) + 65]

            for s in range(N_SPANS):
                qs, qe = 512 * s, 512 * (s + 1)
                nkc = 4 * s + 4
                for h in range(2):
                    po = ps_o.tile([65, 512], F32, tag="po")
                    for g0 in range(0, nkc, 2):
                        kcs = [kc for kc in (g0, g0 + 1) if kc < nkc]
                        pss = ps_s.tile([128, 1024], F32, tag="pss")
                        pe_sb = pbuf.tile([128, 1024], F32, tag="pe")
                        blocks = []
                        off = 0
                        for kc in kcs:
                            qb = max(qs, 128 * kc)
                            w = qe - qb
                            nc.tensor.matmul(
                                pss[:, off : off + w],
                                k_slice(h, kc),
                                q_slice(h, qb, qe),
                                start=True,
                                stop=True,
                            )
                            blocks.append((kc, off, qb, w))
                            off += w
                        # causal mask on diagonal blocks (first 128 cols)
                        for kc, o, qb, w in blocks:
                            if kc >= 4 * s:
                                nc.vector.tensor_add(
                                    out=pss[:, o : o + 128],
                                    in0=pss[:, o : o + 128],
                                    in1=mask_sb,
                                )
                        nc.scalar.activation(
                            out=pe_sb[:, :off],
                            in_=pss[:, :off],
                            func=mybir.ActivationFunctionType.Exp,
                            scale=0.125,
                        )
                        for kc, o, qb, w in blocks:
                            nc.tensor.matmul(
                                po[:, qb - qs : qb - qs + w],
                                v_slice(h, kc),
                                pe_sb[:, o : o + w],
                                start=(kc == 0),
                                stop=(kc == nkc - 1),
                            )
                    # normalize: out[d, q] = po[d, q] / po[64, q]
                    r_sb = nbuf.tile([65, 512], F32, tag="r")
                    nc.vector.reciprocal(out=r_sb[64:65, :], in_=po[64:65, :])
                    pr = ps_r.tile([64, 512], F32, tag="pr")
                    nc.tensor.matmul(
                        pr, ones_sb[64:65, 0:64], r_sb[64:65, :],
                        start=True, stop=True,
                    )
                    rb_sb = nbuf.tile([64, 512], F32, tag="rb")
                    nc.vector.tensor_copy(out=rb_sb, in_=pr)
                    o_sb = nbuf.tile([64, 512], F32, tag="o")
                    nc.vector.tensor_mul(out=o_sb, in0=po[0:64, :], in1=rb_sb)
                    nc.sync.dma_start(out=outT[h, :, qs:qe], in_=o_sb)

    nc.compile()
    return nc


def _get_nc():
    global _NC
    if _NC is None:
        _NC = _build_bass()
    return _NC


def kernel(q, k, v):
    global _LAST_RESULTS
    q = np.asarray(q, dtype=np.float32)
    k = np.asarray(k, dtype=np.float32)
    v = np.asarray(v, dtype=np.float32)
    assert q.shape == (B, H, S, D)

    in_maps = []
    for c in range(N_CORES):
        h0 = HEADS_PER_CORE * c
        qT = np.ascontiguousarray(
            q[0, h0 : h0 + 2].transpose(0, 2, 1).reshape(128, S)
        )
        kT = np.ascontiguousarray(
            k[0, h0 : h0 + 2].transpose(0, 2, 1).reshape(128, S)
        )
        va = np.ones((2, 128, N_CHUNKS, 65), dtype=np.float32)
        va[..., :64] = (
            v[0, h0 : h0 + 2].reshape(2, N_CHUNKS, 128, 64).transpose(0, 2, 1, 3)
        )
        in_maps.append(
            {"qT": qT, "kT": kT, "vaug": va.reshape(2, 128, 65 * N_CHUNKS)}
        )

    nc = _get_nc()
    res = run_bass_kernel_spmd(nc, in_maps, core_ids=list(range(N_CORES)))
    _LAST_RESULTS = res

    out = np.empty((B, H, S, D), dtype=np.float32)
    for c in range(N_CORES):
        ot = res.results[c]["outT"]  # [2, 64, 2048]
        out[0, 2 * c] = ot[0].T
        out[0, 2 * c + 1] = ot[1].T
    return out


# revision 12
# speedup vs baseline: 1.5863x; 1.5863x over previous
"""Causal flash attention for Trainium2, sharded 2 heads/core over 8 cores.

Math per head: out = softmax_causal(Q K^T / sqrt(D)) @ V,  Q/K/V [S=2048, D=64] fp32.

Device layout (per core, heads h0=2c, h1=2c+1):
  qT   [128, 2048]  rows 64h+d = Q[h]^T        (D on partitions, both heads stacked)
  kT   [128, 2048]  same for K
  vaug [2, 128, 1040] vaug[h, p, 65*kc+d] = V[h, 128*kc+p, d], d=64 column is ones
  outT [2, 64, 2048]  out[h]^T (normalized)

Scores are computed transposed (S^T[k, q] = K_chunk @ Q^T) so no transposes are
needed anywhere: softmax denominator comes out of the PV matmul via the ones
column of vaug (psum row 64), and the final division is done by broadcasting
1/denom across partitions with a K=1 matmul against a ones vector.
"""

import os
import sys

import numpy as np

sys.path.insert(0, "/opt/trn_rl_repo")

import concourse.bass as bass
import concourse.bacc as bacc
import concourse.mybir as mybir
import concourse.tile as tile
from concourse.bass_utils import run_bass_kernel_spmd

B, H, S, D = 1, 16, 2048, 64
N_CORES = 8
HEADS_PER_CORE = H // N_CORES  # 2
N_CHUNKS = S // 128  # 16 key chunks per head
N_SPANS = S // 512  # 4 query spans per head
F32 = mybir.dt.float32
R32 = mybir.dt.float32r  # one-pass fp32 matmul mode (TF32-like); 2x PE throughput
NEG = -1.0e30


def _r(ap):
    return ap.bitcast(R32)

_NC = None
_LAST_RESULTS = None


def _build_bass():
    nc = bacc.Bacc("TRN2", target_bir_lowering=False)
    qT = nc.declare_dram_parameter("qT", [128, S], R32, isOutput=False)
    kT = nc.declare_dram_parameter("kT", [128, S], R32, isOutput=False)
    vaug = nc.declare_dram_parameter("vaug", [2, 128, 65 * N_CHUNKS], R32, isOutput=False)
    outT = nc.declare_dram_parameter("outT", [2, 64, S], F32, isOutput=True)

    with tile.TileContext(nc) as tc:
        with (
            tc.tile_pool(name="const", bufs=1) as const,
            tc.tile_pool(name="inbuf", bufs=1) as inbuf,
            tc.tile_pool(name="pbuf", bufs=3) as pbuf,
            tc.tile_pool(name="nbuf", bufs=3) as nbuf,
            tc.tile_pool(name="ps_s", bufs=2, space="PSUM") as ps_s,
            tc.tile_pool(name="ps_o", bufs=2, space="PSUM") as ps_o,
            tc.tile_pool(name="ps_r", bufs=2, space="PSUM") as ps_r,
        ):
            # Additive causal mask for a diagonal 128x128 block:
            # keep score where q >= k, else -1e30.
            mask_sb = const.tile([128, 128], F32)
            nc.gpsimd.memset(mask_sb, 0.0)
            nc.gpsimd.affine_select(
                out=mask_sb,
                in_=mask_sb,
                compare_op=mybir.AluOpType.is_ge,
                fill=NEG,
                base=0,
                pattern=[[1, 128]],  # +q per free element
                channel_multiplier=-1,  # -k per partition
            )
            # Ones vector on partition 64 (K=1 lhsT for the 1/denom broadcast
            # matmul; partition must match the denom row of psum_o).
            ones_sb = const.tile([65, 64], F32)
            nc.vector.memset(ones_sb, 1.0)

            # Input loads, chunked by 512 query/key columns so compute can
            # start before all DMAs land.
            qsb = []
            ksb = []
            for j in range(N_SPANS):
                qt = inbuf.tile([128, 512], R32, tag=f"q{j}")
                nc.sync.dma_start(out=qt, in_=qT[:, 512 * j : 512 * (j + 1)])
                qsb.append(qt)
                kt = inbuf.tile([128, 512], R32, tag=f"k{j}")
                nc.sync.dma_start(out=kt, in_=kT[:, 512 * j : 512 * (j + 1)])
                ksb.append(kt)
            vsb = []
            for h in range(2):
                row = []
                for j in range(N_SPANS):
                    vt = inbuf.tile([128, 4 * 65], R32, tag=f"v{h}{j}")
                    nc.sync.dma_start(
                        out=vt, in_=vaug[h, :, 260 * j : 260 * (j + 1)]
                    )
                    row.append(vt)
                vsb.append(row)

            def k_slice(h, kc):
                # kT chunk [64, 128] for head h: lhsT of the scores matmul.
                return ksb[kc // 4][64 * h : 64 * h + 64, 128 * (kc % 4) : 128 * (kc % 4) + 128]

            def q_slice(h, qs, qe):
                j = qs // 512
                base = 512 * j
                return qsb[j][64 * h : 64 * h + 64, qs - base : qe - base]

            def v_slice(h, kc):
                return vsb[h][kc // 4][:, 65 * (kc % 4) : 65 * (kc % 4) + 65]

            for s in range(N_SPANS):
                qs, qe = 512 * s, 512 * (s + 1)
                nkc = 4 * s + 4
                for h in range(2):
                    po = ps_o.tile([65, 512], F32, tag="po")
                    for g0 in range(0, nkc, 2):
                        kcs = [kc for kc in (g0, g0 + 1) if kc < nkc]
                        pss = ps_s.tile([128, 1024], F32, tag="pss")
                        pe_sb = pbuf.tile([128, 1024], R32, tag="pe")
                        blocks = []
                        off = 0
                        for kc in kcs:
                            qb = max(qs, 128 * kc)
                            w = qe - qb
                            nc.tensor.matmul(
                                pss[:, off : off + w],
                                k_slice(h, kc),
                                q_slice(h, qb, qe),
                                start=True,
                                stop=True,
                            )
                            blocks.append((kc, off, qb, w))
                            off += w
                        # causal mask on diagonal blocks (first 128 cols)
                        for kc, o, qb, w in blocks:
                            if kc >= 4 * s:
                                nc.vector.tensor_add(
                                    out=pss[:, o : o + 128],
                                    in0=pss[:, o : o + 128],
                                    in1=mask_sb,
                                )
                        nc.scalar.activation(
                            out=pe_sb[:, :off],
                            in_=pss[:, :off],
                            func=mybir.ActivationFunctionType.Exp,
                            scale=0.125,
                        )
                        for kc, o, qb, w in blocks:
                            nc.tensor.matmul(
                                po[:, qb - qs : qb - qs + w],
                                v_slice(h, kc),
                                pe_sb[:, o : o + w],
                                start=(kc == 0),
                                stop=(kc == nkc - 1),
                            )
                    # normalize: out[d, q] = po[d, q] / po[64, q].
                    # 1/d computed as exp(-ln(d)) on ACT (DVE reciprocal is
                    # ~8 cyc/elem; ACT is ~1/cyc and Log+Exp share one table
                    # set). Then broadcast across partitions via a K=1 matmul.
                    r_sb = nbuf.tile([65, 512], F32, tag="r")
                    nc.scalar.activation(
                        out=r_sb[64:65, :], in_=po[64:65, :],
                        func=mybir.ActivationFunctionType.Ln,
                    )
                    nc.scalar.activation(
                        out=r_sb[64:65, :], in_=r_sb[64:65, :],
                        func=mybir.ActivationFunctionType.Exp, scale=-1.0,
                    )
                    pr = ps_r.tile([64, 512], F32, tag="pr")
                    nc.tensor.matmul(
                        pr, ones_sb[64:65, 0:64], r_sb[64:65, :],
                        start=True, stop=True,
                    )
                    rb_sb = nbuf.tile([64, 512], F32, tag="rb")
                    nc.vector.tensor_copy(out=rb_sb, in_=pr)
                    o_sb = nbuf.tile([64, 512], F32, tag="o")
                    nc.vector.tensor_mul(out=o_sb, in0=po[0:64, :], in1=rb_sb)
                    nc.sync.dma_start(out=outT[h, :, qs:qe], in_=o_sb)

    nc.compile()
    return nc


def _get_nc():
    global _NC
    if _NC is None:
        _NC = _build_bass()
    return _NC


def kernel(q, k, v):
    global _LAST_RESULTS
    q = np.asarray(q, dtype=np.float32)
    k = np.asarray(k, dtype=np.float32)
    v = np.asarray(v, dtype=np.float32)
    assert q.shape == (B, H, S, D)

    in_maps = []
    for c in range(N_CORES):
        h0 = HEADS_PER_CORE * c
        qT = np.ascontiguousarray(
            q[0, h0 : h0 + 2].transpose(0, 2, 1).reshape(128, S)
        )
        kT = np.ascontiguousarray(
            k[0, h0 : h0 + 2].transpose(0, 2, 1).reshape(128, S)
        )
        va = np.ones((2, 128, N_CHUNKS, 65), dtype=np.float32)
        va[..., :64] = (
            v[0, h0 : h0 + 2].reshape(2, N_CHUNKS, 128, 64).transpose(0, 2, 1, 3)
        )
        in_maps.append(
            {"qT": qT, "kT": kT, "vaug": va.reshape(2, 128, 65 * N_CHUNKS)}
        )

    nc = _get_nc()
    res = run_bass_kernel_spmd(nc, in_maps, core_ids=list(range(N_CORES)))
    _LAST_RESULTS = res

    out = np.empty((B, H, S, D), dtype=np.float32)
    for c in range(N_CORES):
        ot = res.results[c]["outT"]  # [2, 64, 2048]
        out[0, 2 * c] = ot[0].T
        out[0, 2 * c + 1] = ot[1].T
    return out
